# revision 10
# baseline (speedup 1.0000x reference)
"""AdaptiveTopK Trainium2 kernel (8 NeuronCores, SPMD data-parallel).

Problem: indexer_scores [4, 4096, 4096] f32.
Returns (selected_indices [4, 4096, 1024] int32, k_values [4, 4096] int32)
matching:
    variance = var(x, axis=-1, ddof=1)
    k_values = clip(512*(1+0.1*softplus(variance)), 128, 1024).astype(int32)
    selected_indices = lax.top_k(x, 1024)[1]   # desc by value, ties by index

Sharding: rows (B*L = 16384) split evenly across 8 cores (pure data
parallel, no collectives). Each core processes 2048 rows of 4096 as 16
tiles of [128 partitions x 4096].
"""

import os

import numpy as np

import concourse.bass as bass
import concourse.mybir as mybir
import concourse.tile as tile
from concourse.bass_utils import run_bass_kernel_spmd

AF = mybir.ActivationFunctionType
ALU = mybir.AluOpType
DT = mybir.dt

B, L = 4, 4096
K = 1024
N_CORES = 8
ROWS_PER_CORE = (B * L) // N_CORES  # 2048
P = 128

NEG_INF = -3.4028235e38  # -FLT_MAX


def build_bass(rows: int = ROWS_PER_CORE) -> bass.Bass:
    """Build the per-core Bass graph: x [rows, 4096] -> idx [rows, 1024] u32,
    kv [rows, 1] i32."""
    assert rows % P == 0
    n_tiles = rows // P

    nc = bass.Bass()
    x_ext = nc.declare_dram_parameter("x", [rows, L], DT.float32, isOutput=False)
    idx_ext = nc.declare_dram_parameter("idx", [rows, K], DT.uint32, isOutput=True)
    kv_ext = nc.declare_dram_parameter("kv", [rows, 1], DT.int32, isOutput=True)

    with tile.TileContext(nc) as tc:
        with (
            tc.tile_pool(name="data", bufs=2) as data_pool,
            tc.tile_pool(name="scratch", bufs=2) as scratch_pool,
            tc.tile_pool(name="out", bufs=2) as out_pool,
            tc.tile_pool(name="small", bufs=2) as small_pool,
        ):
            for t in range(n_tiles):
                rs = t * P
                data = data_pool.tile([P, L], DT.float32)
                nc.sync.dma_start(data[:], x_ext[rs : rs + P, :])

                # ---- stats -> k_values (one-pass variance; mu ~ 0 so no
                # cancellation trouble) ----
                s1 = small_pool.tile([P, 1], DT.float32, tag="s1")
                nc.vector.reduce_sum(s1[:], data[:], axis=mybir.AxisListType.X)
                sq = scratch_pool.tile([P, L], DT.float32, tag="sq")
                nc.vector.tensor_tensor(sq[:], data[:], data[:], op=ALU.mult)
                s2 = small_pool.tile([P, 1], DT.float32, tag="s2")
                nc.vector.reduce_sum(s2[:], sq[:], axis=mybir.AxisListType.X)
                t1 = small_pool.tile([P, 1], DT.float32, tag="t1")
                nc.vector.tensor_tensor(t1[:], s1[:], s1[:], op=ALU.mult)
                nc.vector.tensor_scalar_mul(t1[:], t1[:], 1.0 / L)
                var = small_pool.tile([P, 1], DT.float32, tag="var")
                nc.vector.tensor_tensor(var[:], s2[:], t1[:], op=ALU.subtract)
                nc.vector.tensor_scalar_mul(var[:], var[:], 1.0 / (L - 1))

                # softplus(var) = ln(1 + exp(var))
                ev = small_pool.tile([P, 1], DT.float32, tag="ev")
                nc.scalar.activation(ev[:], var[:], AF.Exp)
                nc.vector.tensor_scalar_add(ev[:], ev[:], 1.0)
                sp = small_pool.tile([P, 1], DT.float32, tag="sp")
                nc.scalar.activation(sp[:], ev[:], AF.Ln)

                # k_adaptive = 512 + 51.2 * sp, clipped to [128, 1024]
                ka = small_pool.tile([P, 1], DT.float32, tag="ka")
                nc.vector.tensor_scalar(
                    ka[:], sp[:], 51.2, 512.0, op0=ALU.mult, op1=ALU.add
                )
                nc.vector.tensor_scalar_max(ka[:], ka[:], 128.0)
                nc.vector.tensor_scalar_min(ka[:], ka[:], 1024.0)

                # truncate toward zero, robust to convert rounding mode:
                # ki = int(ka); if float(ki) > ka: ki -= 1
                ki = small_pool.tile([P, 1], DT.int32, tag="ki")
                nc.vector.tensor_copy(ki[:], ka[:])
                kf = small_pool.tile([P, 1], DT.float32, tag="kf")
                nc.vector.tensor_copy(kf[:], ki[:])
                corrf = small_pool.tile([P, 1], DT.float32, tag="corrf")
                nc.vector.tensor_tensor(corrf[:], kf[:], ka[:], op=ALU.is_gt)
                corri = small_pool.tile([P, 1], DT.int32, tag="corri")
                nc.vector.tensor_copy(corri[:], corrf[:])
                nc.vector.tensor_tensor(ki[:], ki[:], corri[:], op=ALU.subtract)
                nc.sync.dma_start(kv_ext[rs : rs + P, :], ki[:])

                # ---- top-1024 extraction, 8 at a time ----
                idx_tile = out_pool.tile([P, K], DT.uint32, tag="idx")
                vals8 = small_pool.tile([P, 8], DT.float32, tag="vals8")
                for r in range(K // 8):
                    nc.vector.max(out=vals8[:], in_=data[:])
                    nc.vector.max_index(
                        out=idx_tile[:, r * 8 : (r + 1) * 8],
                        in_max=vals8[:],
                        in_values=data[:],
                    )
                    nc.vector.match_replace(
                        out=data[:],
                        in_to_replace=vals8[:],
                        in_values=data[:],
                        imm_value=NEG_INF,
                    )
                nc.sync.dma_start(idx_ext[rs : rs + P, :], idx_tile[:])

    _split_fat_waits(nc)
    return nc


_MAX_SYNC_WAITS = 1


def _split_fat_waits(nc: bass.Bass) -> None:
    """Walrus caps semaphore waits per instruction (~4 on CTRL structs).
    Tile's kernel-tail drain can exceed that when many DMA-queue sems are
    live. Offload excess waits onto same-engine NoOps inserted just before
    the offending instruction (program order preserves semantics)."""
    cnt = 0
    for f in nc.m.functions:
        for bb in f.blocks:
            insts = bb.instructions
            i = 0
            while i < len(insts):
                inst = insts[i]
                si = inst.sync_info
                if si is not None and si.on_wait and len(si.on_wait) > _MAX_SYNC_WAITS:
                    waits = list(si.on_wait)
                    keep = waits[-_MAX_SYNC_WAITS:]
                    rest = waits[:-_MAX_SYNC_WAITS]
                    pos = i
                    for j in range(0, len(rest), _MAX_SYNC_WAITS):
                        n = mybir.InstNoOp(name=f"I-waitsplit-{cnt}")
                        cnt += 1
                        n.engine = inst.engine
                        n.sync_info = mybir.SyncInfo(
                            on_wait=rest[j : j + _MAX_SYNC_WAITS], on_update=[]
                        )
                        insts.insert(pos, n)
                        pos += 1
                        i += 1
                    inst.sync_info = mybir.SyncInfo(
                        on_wait=keep, on_update=list(si.on_update)
                    )
                i += 1


_NC_CACHE: dict[int, bass.Bass] = {}


def _get_nc(rows: int) -> bass.Bass:
    if rows not in _NC_CACHE:
        _NC_CACHE[rows] = build_bass(rows)
    return _NC_CACHE[rows]


def kernel(indexer_scores: np.ndarray):
    x = np.ascontiguousarray(np.asarray(indexer_scores, dtype=np.float32))
    assert x.shape == (B, L, L), x.shape
    flat = x.reshape(B * L, L)
    shards = [
        np.ascontiguousarray(flat[i * ROWS_PER_CORE : (i + 1) * ROWS_PER_CORE])
        for i in range(N_CORES)
    ]
    in_maps = [{"x": s} for s in shards]

    nc = _get_nc(ROWS_PER_CORE)
    res = run_bass_kernel_spmd(nc, in_maps, core_ids=list(range(N_CORES)), trace=False)

    if res.exec_time_ns is not None:
        print(f"HW exec time: {res.exec_time_ns} ns")

    idx = np.concatenate(
        [np.asarray(r["idx"]).astype(np.int64).astype(np.int32) for r in res.results],
        axis=0,
    ).reshape(B, L, K)
    kv = np.concatenate(
        [np.asarray(r["kv"]).reshape(-1).astype(np.int32) for r in res.results],
        axis=0,
    ).reshape(B, L)
    return idx, kv


def bench(indexer_scores: np.ndarray, iters: int = 5) -> float:
    """Time the on-device NEFF execution (min over iters, seconds).

    Replicates bass2jax.run_bass_via_pjrt's shard_map structure but keeps
    the jitted callable and pre-transferred device inputs so repeated
    calls measure device execution (plus dispatch) rather than transfers.
    """
    import time

    import jax
    from jax.experimental.shard_map import shard_map
    from jax.sharding import Mesh, PartitionSpec

    from concourse import bass2jax, mybir as _mb

    x = np.ascontiguousarray(np.asarray(indexer_scores, dtype=np.float32))
    flat = x.reshape(B * L, L)

    nc = _get_nc(ROWS_PER_CORE)
    bass2jax.install_neuronx_cc_hook()

    partition_name = (
        nc.partition_id_tensor.name if nc.partition_id_tensor else None
    )
    in_names, out_names, out_avals, zero_outs = [], [], [], []
    for alloc in nc.m.functions[0].allocations:
        if not isinstance(alloc, _mb.MemoryLocationSet):
            continue
        name = alloc.memorylocations[0].name
        if alloc.kind == "ExternalInput":
            if name != partition_name:
                in_names.append(name)
        elif alloc.kind == "ExternalOutput":
            out_names.append(name)
            shape = tuple(alloc.tensor_shape)
            dtype = _mb.dt.np(alloc.dtype)
            out_avals.append(jax.core.ShapedArray(shape, dtype))
            zero_outs.append(np.zeros(shape, dtype))
    n_params = len(in_names)
    all_names = in_names + out_names

    def _body(*args):
        operands = list(args)
        names = list(all_names)
        if partition_name is not None:
            operands.append(bass2jax.partition_id_tensor())
            names.append(partition_name)
        outs = bass2jax._bass_exec_p.bind(
            *operands,
            out_avals=tuple(out_avals),
            in_names=tuple(names),
            out_names=tuple(out_names),
            lowering_input_output_aliases=(),
            sim_require_finite=True,
            sim_require_nnan=True,
            nc=nc,
        )
        return tuple(outs)

    devices = jax.devices()[:N_CORES]
    mesh = Mesh(np.asarray(devices), ("core",))
    in_specs = (PartitionSpec("core"),) * (n_params + len(out_names))
    out_specs = (PartitionSpec("core"),) * len(out_names)
    fn = jax.jit(
        shard_map(_body, mesh=mesh, in_specs=in_specs, out_specs=out_specs,
                  check_rep=False),
        keep_unused=True,
    )
    ins = {"x": flat}
    concat_in = [ins[n] for n in in_names]
    concat_zeros = [
        np.zeros((N_CORES * z.shape[0], *z.shape[1:]), z.dtype) for z in zero_outs
    ]
    args = concat_in + concat_zeros
    sharding = jax.sharding.NamedSharding(mesh, PartitionSpec("core"))
    dev_args = [jax.device_put(a, sharding) for a in args]

    # warmup (compile + first exec)
    out = fn(*dev_args)
    jax.block_until_ready(out)
    best = float("inf")
    for _ in range(iters):
        t0 = time.perf_counter()
        out = fn(*dev_args)
        jax.block_until_ready(out)
        best = min(best, time.perf_counter() - t0)
    return best


# revision 11
# speedup vs baseline: 1.3089x; 1.3089x over previous
"""AdaptiveTopK Trainium2 kernel (8 NeuronCores, SPMD data-parallel).

Problem: indexer_scores [4, 4096, 4096] f32.
Returns (selected_indices [4, 4096, 1024] int32, k_values [4, 4096] int32)
matching:
    variance = var(x, axis=-1, ddof=1)
    k_values = clip(512*(1+0.1*softplus(variance)), 128, 1024).astype(int32)
    selected_indices = lax.top_k(x, 1024)[1]   # desc by value, ties by index

Sharding: rows (B*L = 16384) split evenly across 8 cores (pure data
parallel, no collectives). Each core processes 2048 rows of 4096 as 16
tiles of [128 partitions x 4096].
"""

import os

import numpy as np

import concourse.bass as bass
import concourse.mybir as mybir
import concourse.tile as tile
from concourse.bass_utils import run_bass_kernel_spmd

AF = mybir.ActivationFunctionType
ALU = mybir.AluOpType
DT = mybir.dt

B, L = 4, 4096
K = 1024
N_CORES = 8
ROWS_PER_CORE = (B * L) // N_CORES  # 2048
P = 128

NEG_INF = -3.4028235e38  # -FLT_MAX


def build_bass(rows: int = ROWS_PER_CORE, split_waits: bool = True) -> bass.Bass:
    """Build the per-core Bass graph: x [rows, 4096] -> idx [rows, 1024] u32,
    kv [rows, 1] i32."""
    assert rows % P == 0
    n_tiles = rows // P

    nc = bass.Bass()
    x_ext = nc.declare_dram_parameter("x", [rows, L], DT.float32, isOutput=False)
    idx_ext = nc.declare_dram_parameter("idx", [rows, K], DT.uint32, isOutput=True)
    kv_ext = nc.declare_dram_parameter("kv", [rows, 1], DT.int32, isOutput=True)

    with tile.TileContext(nc) as tc:
        with (
            tc.tile_pool(name="data", bufs=2) as data_pool,
            tc.tile_pool(name="scratch", bufs=2) as scratch_pool,
            tc.tile_pool(name="out", bufs=2) as out_pool,
            tc.tile_pool(name="small", bufs=2) as small_pool,
        ):
            for t in range(n_tiles):
                rs = t * P
                data = data_pool.tile([P, L], DT.float32)
                nc.sync.dma_start(data[:], x_ext[rs : rs + P, :])

                # ---- stats -> k_values (one-pass variance; mu ~ 0 so no
                # cancellation trouble) ----
                s1 = small_pool.tile([P, 1], DT.float32, tag="s1")
                nc.vector.reduce_sum(s1[:], data[:], axis=mybir.AxisListType.X)
                sq = scratch_pool.tile([P, L], DT.float32, tag="sq")
                nc.vector.tensor_tensor(sq[:], data[:], data[:], op=ALU.mult)
                s2 = small_pool.tile([P, 1], DT.float32, tag="s2")
                nc.vector.reduce_sum(s2[:], sq[:], axis=mybir.AxisListType.X)
                t1 = small_pool.tile([P, 1], DT.float32, tag="t1")
                nc.vector.tensor_tensor(t1[:], s1[:], s1[:], op=ALU.mult)
                nc.vector.tensor_scalar_mul(t1[:], t1[:], 1.0 / L)
                var = small_pool.tile([P, 1], DT.float32, tag="var")
                nc.vector.tensor_tensor(var[:], s2[:], t1[:], op=ALU.subtract)
                nc.vector.tensor_scalar_mul(var[:], var[:], 1.0 / (L - 1))

                # softplus(var) = ln(1 + exp(var))
                ev = small_pool.tile([P, 1], DT.float32, tag="ev")
                nc.scalar.activation(ev[:], var[:], AF.Exp)
                nc.vector.tensor_scalar_add(ev[:], ev[:], 1.0)
                sp = small_pool.tile([P, 1], DT.float32, tag="sp")
                nc.scalar.activation(sp[:], ev[:], AF.Ln)

                # k_adaptive = 512 + 51.2 * sp, clipped to [128, 1024]
                ka = small_pool.tile([P, 1], DT.float32, tag="ka")
                nc.vector.tensor_scalar(
                    ka[:], sp[:], 51.2, 512.0, op0=ALU.mult, op1=ALU.add
                )
                nc.vector.tensor_scalar_max(ka[:], ka[:], 128.0)
                nc.vector.tensor_scalar_min(ka[:], ka[:], 1024.0)

                # truncate toward zero, robust to convert rounding mode:
                # ki = int(ka); if float(ki) > ka: ki -= 1
                ki = small_pool.tile([P, 1], DT.int32, tag="ki")
                nc.vector.tensor_copy(ki[:], ka[:])
                kf = small_pool.tile([P, 1], DT.float32, tag="kf")
                nc.vector.tensor_copy(kf[:], ki[:])
                corrf = small_pool.tile([P, 1], DT.float32, tag="corrf")
                nc.vector.tensor_tensor(corrf[:], kf[:], ka[:], op=ALU.is_gt)
                corri = small_pool.tile([P, 1], DT.int32, tag="corri")
                nc.vector.tensor_copy(corri[:], corrf[:])
                nc.vector.tensor_tensor(ki[:], ki[:], corri[:], op=ALU.subtract)
                nc.sync.dma_start(kv_ext[rs : rs + P, :], ki[:])

                # ---- top-1024 extraction, 8 at a time ----
                idx_tile = out_pool.tile([P, K], DT.uint32, tag="idx")
                vals8 = small_pool.tile([P, 8], DT.float32, tag="vals8")
                for r in range(K // 8):
                    nc.vector.max(out=vals8[:], in_=data[:])
                    nc.vector.max_index(
                        out=idx_tile[:, r * 8 : (r + 1) * 8],
                        in_max=vals8[:],
                        in_values=data[:],
                    )
                    nc.vector.match_replace(
                        out=data[:],
                        in_to_replace=vals8[:],
                        in_values=data[:],
                        imm_value=NEG_INF,
                    )
                nc.sync.dma_start(idx_ext[rs : rs + P, :], idx_tile[:])

    if split_waits:
        _split_fat_waits(nc)
    return nc


_MAX_SYNC_WAITS = 1


def _split_fat_waits(nc: bass.Bass) -> None:
    """Walrus caps semaphore waits per instruction (~4 on CTRL structs).
    Tile's kernel-tail drain can exceed that when many DMA-queue sems are
    live. Offload excess waits onto same-engine NoOps inserted just before
    the offending instruction (program order preserves semantics)."""
    cnt = 0
    for f in nc.m.functions:
        for bb in f.blocks:
            insts = bb.instructions
            i = 0
            while i < len(insts):
                inst = insts[i]
                si = inst.sync_info
                if si is not None and si.on_wait and len(si.on_wait) > _MAX_SYNC_WAITS:
                    waits = list(si.on_wait)
                    keep = waits[-_MAX_SYNC_WAITS:]
                    rest = waits[:-_MAX_SYNC_WAITS]
                    pos = i
                    for j in range(0, len(rest), _MAX_SYNC_WAITS):
                        n = mybir.InstNoOp(name=f"I-waitsplit-{cnt}")
                        cnt += 1
                        n.engine = inst.engine
                        n.sync_info = mybir.SyncInfo(
                            on_wait=rest[j : j + _MAX_SYNC_WAITS], on_update=[]
                        )
                        insts.insert(pos, n)
                        pos += 1
                        i += 1
                    inst.sync_info = mybir.SyncInfo(
                        on_wait=keep, on_update=list(si.on_update)
                    )
                i += 1


_NC_CACHE: dict[int, bass.Bass] = {}


def _get_nc(rows: int) -> bass.Bass:
    if rows not in _NC_CACHE:
        _NC_CACHE[rows] = build_bass(rows)
    return _NC_CACHE[rows]


def kernel(indexer_scores: np.ndarray):
    x = np.ascontiguousarray(np.asarray(indexer_scores, dtype=np.float32))
    assert x.shape == (B, L, L), x.shape
    flat = x.reshape(B * L, L)
    shards = [
        np.ascontiguousarray(flat[i * ROWS_PER_CORE : (i + 1) * ROWS_PER_CORE])
        for i in range(N_CORES)
    ]
    in_maps = [{"x": s} for s in shards]

    nc = _get_nc(ROWS_PER_CORE)
    res = run_bass_kernel_spmd(nc, in_maps, core_ids=list(range(N_CORES)), trace=False)

    if res.exec_time_ns is not None:
        print(f"HW exec time: {res.exec_time_ns} ns")

    idx = np.concatenate(
        [np.asarray(r["idx"]).astype(np.int64).astype(np.int32) for r in res.results],
        axis=0,
    ).reshape(B, L, K)
    kv = np.concatenate(
        [np.asarray(r["kv"]).reshape(-1).astype(np.int32) for r in res.results],
        axis=0,
    ).reshape(B, L)
    return idx, kv


def bench(indexer_scores: np.ndarray, iters: int = 5) -> float:
    """Time the on-device NEFF execution (min over iters, seconds).

    Replicates bass2jax.run_bass_via_pjrt's shard_map structure but keeps
    the jitted callable and pre-transferred device inputs so repeated
    calls measure device execution (plus dispatch) rather than transfers.
    """
    import time

    import jax
    from jax.experimental.shard_map import shard_map
    from jax.sharding import Mesh, PartitionSpec

    from concourse import bass2jax, mybir as _mb

    x = np.ascontiguousarray(np.asarray(indexer_scores, dtype=np.float32))
    flat = x.reshape(B * L, L)

    nc = _get_nc(ROWS_PER_CORE)
    bass2jax.install_neuronx_cc_hook()

    partition_name = (
        nc.partition_id_tensor.name if nc.partition_id_tensor else None
    )
    in_names, out_names, out_avals, zero_outs = [], [], [], []
    for alloc in nc.m.functions[0].allocations:
        if not isinstance(alloc, _mb.MemoryLocationSet):
            continue
        name = alloc.memorylocations[0].name
        if alloc.kind == "ExternalInput":
            if name != partition_name:
                in_names.append(name)
        elif alloc.kind == "ExternalOutput":
            out_names.append(name)
            shape = tuple(alloc.tensor_shape)
            dtype = _mb.dt.np(alloc.dtype)
            out_avals.append(jax.core.ShapedArray(shape, dtype))
            zero_outs.append(np.zeros(shape, dtype))
    n_params = len(in_names)
    all_names = in_names + out_names

    def _body(*args):
        operands = list(args)
        names = list(all_names)
        if partition_name is not None:
            operands.append(bass2jax.partition_id_tensor())
            names.append(partition_name)
        outs = bass2jax._bass_exec_p.bind(
            *operands,
            out_avals=tuple(out_avals),
            in_names=tuple(names),
            out_names=tuple(out_names),
            lowering_input_output_aliases=(),
            sim_require_finite=True,
            sim_require_nnan=True,
            nc=nc,
        )
        return tuple(outs)

    devices = jax.devices()[:N_CORES]
    mesh = Mesh(np.asarray(devices), ("core",))
    in_specs = (PartitionSpec("core"),) * (n_params + len(out_names))
    out_specs = (PartitionSpec("core"),) * len(out_names)
    fn = jax.jit(
        shard_map(_body, mesh=mesh, in_specs=in_specs, out_specs=out_specs,
                  check_rep=False),
        keep_unused=True,
    )
    ins = {"x": flat}
    concat_in = [ins[n] for n in in_names]
    concat_zeros = [
        np.zeros((N_CORES * z.shape[0], *z.shape[1:]), z.dtype) for z in zero_outs
    ]
    args = concat_in + concat_zeros
    sharding = jax.sharding.NamedSharding(mesh, PartitionSpec("core"))
    dev_args = [jax.device_put(a, sharding) for a in args]

    # warmup (compile + first exec)
    out = fn(*dev_args)
    jax.block_until_ready(out)
    best = float("inf")
    for _ in range(iters):
        t0 = time.perf_counter()
        out = fn(*dev_args)
        jax.block_until_ready(out)
        best = min(best, time.perf_counter() - t0)
    return best
